# revision 55
# baseline (speedup 1.0000x reference)
"""LocalLinear (per-position 1D conv, K=8) Trainium2 Bass kernel.

Y[n, p] = sum_k X[n, p+k] * W[p, k, 0] + b[p, 0],  X right-padded by K-1.

Strategy: shard the position axis P across the 8 cores (2500 positions each,
with a 7-row halo). The kernel is HBM-bandwidth bound (inputs + outputs are
~160MB vs ~2.9TB/s chip HBM), so all bulk traffic is fp16 with fp32 PSUM
accumulation (end-to-end max rel err ~5.5e-4, far inside the 2e-2 gate).

Per core, positions are processed in chunks of CW=121. One fused fp16 tile
per chunk holds both operands: cols 0..1023 are X^T rows p0..p0+cw+6
(cw+7 <= 128 partitions), cols 1024..1024+cw-1 are the banded stationary
matrix B with B[j+k, j] = W[p0+j, k]. One fp16 matmul per (chunk, 512-col
half of N) computes Y^T[j, n] = sum_q B[q, j] * X^T[p0+q, n] into fp32 PSUM.
The PSUM->SBUF drain casts to fp16 and adds the (fp32) bias b[p0+j] as a
per-partition scalar: DVE tensor_scalar_add for half 0, ACT activation
Identity-with-bias for half 1. Input tiles are prefetched in full across
both HWDGE queues (SP: even chunks, ACT: odd chunks, each with its own
completion semaphore, next-DMA race guard, and trailing sentinel). gpsimd
(SWDGE) DMAs fp16 Y^T out for chunks 0..NCH-2; the last chunk's output is
issued from ACT's HWDGE so its DGE delay is not exposed after the pool
drains. The host upcasts and transposes back.
"""

import numpy as np

N = 1024
P = 20000
K = 8
NCORES = 8
PPC = P // NCORES  # positions per core
CW = 121  # output columns per chunk (CW + K - 1 = 128 partitions)
CHUNKS = [(i * CW, min(CW, PPC - i * CW)) for i in range((PPC + CW - 1) // CW)]
NCH = len(CHUNKS)  # 21, last chunk cw=80
XCOLS = N  # X^T columns per tile
TW = XCOLS + 128  # fused tile width (X cols + banded-W cols, 64B aligned)
NB = NCH  # fused input tile SBUF buffers: full prefetch, no slot reuse
PB = 4  # PSUM buffers (bank pairs; 4 x 4KB = all 8 banks). With uint8
# outputs the end-game is compute-chain paced, so deeper PSUM decoupling
# of matmuls from the PSUM->SBUF drains shortens the exposed tail.
YB = NCH  # y SBUF buffers: one per chunk, no reuse coupling
HALF = 512

_CACHE = {}


def _build_bass():
    import concourse.bass as bass
    from concourse import mybir

    f16 = mybir.dt.float16
    f32 = mybir.dt.float32
    u8 = mybir.dt.uint8
    nc = bass.Bass()
    xin_d = nc.dram_tensor("xin", [NCH, 128, TW], f16, kind="ExternalInput")
    # per chunk: col c = 1/s_p (output quant scale), col NCH+c = b/s_p + 128
    bvec_d = nc.dram_tensor("bvec", [128, 2 * NCH], f32, kind="ExternalInput")
    # Y^T quantized to uint8 with per-position scales s_p chosen from the
    # rigorous bound |Y[n,p]| <= sum_k |W[p,k]|*max|X| + |b_p| (never
    # saturates); halves the output traffic vs fp16 at ~1.4e-2 rel err.
    yt_d = nc.dram_tensor("yt", [PPC, N], u8, kind="ExternalOutput")

    with (
        nc.sbuf_tensor("bvec_s", [128, 2 * NCH], f32) as bvec_s,
        nc.sbuf_tensor("x_s", [128, NB * TW], f16) as x_s,
        nc.sbuf_tensor("y_s", [128, YB * N], u8) as y_s,
        nc.psum_tensor("ps", [128, PB * N], f32) as ps,
        nc.semaphore("s_b") as s_b,
        nc.semaphore("s_in") as s_in,
        nc.semaphore("s_ino") as s_ino,
        nc.semaphore("s_pe") as s_pe,
        nc.semaphore("s_dve") as s_dve,
        nc.semaphore("s_act") as s_act,
        nc.semaphore("s_out") as s_out,
        nc.Block() as block,
    ):

        def load_chunk(eng, c, sem):
            cs, cw = CHUNKS[c]
            rows = cw + K - 1
            xs = (c % NB) * TW
            w = XCOLS + cw  # skip unused pad columns
            eng.dma_start(
                out=x_s[0:rows, xs : xs + w], in_=xin_d[c, 0:rows, 0:w]
            ).then_inc(sem, 16)

        # Inputs are split across BOTH HWDGE queues (SP: even chunks,
        # ACT: odd chunks) so throughput holds whether the DMA engines are
        # pooled across queues or capped per queue. Each queue gets its own
        # completion semaphore + tiny trailing sentinel, preserving the
        # per-queue "next DMA done => this one landed" race guard.
        @block.sync
        def _(sync):
            for c in range(0, NCH, 2):
                load_chunk(sync, c, s_in)
            sync.dma_start(
                out=bvec_s[0:1, 0 : 2 * NCH], in_=bvec_d[0:1]
            ).then_inc(s_in, 16)
            # Odd-chunk outputs from SP's HWDGE (issue 625ns): with uint8
            # outputs (344ns transfers) a single SWDGE issuer's 1035ns launch
            # cadence paces the tail. SP is idle after its input issues; all
            # its inputs are already queued, so these waits cannot starve
            # them. gpsimd keeps even chunks; ACT takes the last chunk.
            for c in range(1, NCH, 2):
                cs, cw = CHUNKS[c]
                sync.wait_ge(s_dve, c + 1)
                sync.wait_ge(s_act, c + 1)
                ys = (c % YB) * N
                sync.dma_start(
                    out=yt_d[cs : cs + cw, :], in_=y_s[0:cw, ys : ys + N]
                ).then_inc(s_out, 16)

        @block.tensor
        def _(tensor):
            for c in range(NCH):
                cs, cw = CHUNKS[c]
                rows = cw + K - 1
                # Wait for the NEXT same-queue DMA: the completion inc of
                # chunk c's own DMA can fire before its last writes are
                # visible to PE (observed as partition-band corruption in
                # matmul half 0). Per-queue completions are in order, so
                # next-done => c landed a full transfer + sem-prop earlier.
                if c % 2 == 0:
                    tensor.wait_ge(s_in, 16 * (c // 2 + 2))
                else:
                    tensor.wait_ge(s_ino, 16 * ((c - 1) // 2 + 2))
                if c >= PB:
                    tensor.wait_ge(s_dve, c - PB + 1)
                    tensor.wait_ge(s_act, c - PB + 1)
                xs = (c % NB) * TW
                pp = (c % PB) * N
                lhsT = x_s[0:rows, xs + XCOLS : xs + XCOLS + cw]
                tensor.matmul(
                    ps[0:cw, pp : pp + HALF],
                    lhsT,
                    x_s[0:rows, xs : xs + HALF],
                    start=True,
                    stop=True,
                )
                # drain per half: signals after the PSUM writes land, and
                # lets DVE start on half 0 while PE runs half 1
                tensor.drain().then_inc(s_pe, 1)
                tensor.matmul(
                    ps[0:cw, pp + HALF : pp + N],
                    lhsT,
                    x_s[0:rows, xs + HALF : xs + XCOLS],
                    start=True,
                    stop=True,
                )
                tensor.drain().then_inc(s_pe, 1)

        @block.vector
        def _(vector):
            vector.wait_ge(s_b, 16)
            for c in range(NCH):
                cs, cw = CHUNKS[c]
                vector.wait_ge(s_pe, 2 * c + 1)
                pp = (c % PB) * N
                ys = (c % YB) * N
                vector.tensor_scalar(
                    y_s[0:cw, ys : ys + HALF],
                    ps[0:cw, pp : pp + HALF],
                    bvec_s[0:cw, c : c + 1],
                    bvec_s[0:cw, NCH + c : NCH + c + 1],
                    mybir.AluOpType.mult,
                    mybir.AluOpType.add,
                ).then_inc(s_dve, 1)

        @block.scalar
        def _(scalar):
            # ACT's HWDGE queue: bias vec first (needed by the first copies),
            # then the first 4 odd-chunk inputs up-front (~2.5us, still
            # before the first copy is runnable), then the rest interleaved
            # with the copy loop at a 4-odd-chunk lookahead so neither the
            # inputs nor the trailing sentinel ever gate on copy progress.
            load_chunk(scalar, 1, s_ino)
            scalar.dma_start(out=bvec_s[:], in_=bvec_d[:]).then_inc(s_b, 16)
            for oc in (3, 5, 7):
                load_chunk(scalar, oc, s_ino)
            for c in range(NCH):
                oc = 2 * c + 9
                if oc < NCH:
                    load_chunk(scalar, oc, s_ino)
                elif oc == NCH:
                    scalar.dma_start(
                        out=bvec_s[1:2, 0 : 2 * NCH], in_=bvec_d[1:2]
                    ).then_inc(s_ino, 16)
                cs, cw = CHUNKS[c]
                if c == 0:
                    scalar.wait_ge(s_b, 16)
                scalar.wait_ge(s_pe, 2 * c + 2)
                pp = (c % PB) * N
                ys = (c % YB) * N
                scalar.activation(
                    y_s[0:cw, ys + HALF : ys + N],
                    ps[0:cw, pp + HALF : pp + N],
                    mybir.ActivationFunctionType.Identity,
                    bias=bvec_s[0:cw, NCH + c : NCH + c + 1],
                    scale=bvec_s[0:cw, c : c + 1],
                ).then_inc(s_act, 1)
            # Last output via ACT's HWDGE: the SWDGE FIFO triggers entries in
            # order, so on gpsimd the final output's DGE delay lands after the
            # pool drains (~330ns of exposed tail). ACT's queue is idle by
            # now and triggers independently.
            lc, lw = CHUNKS[NCH - 1]
            scalar.wait_ge(s_dve, NCH)
            scalar.dma_start(
                out=yt_d[lc : lc + lw, :],
                in_=y_s[0:lw, ((NCH - 1) % YB) * N : ((NCH - 1) % YB) * N + N],
            ).then_inc(s_out, 16)

        @block.gpsimd
        def _(g):
            for c in range(0, NCH - 1, 2):
                cs, cw = CHUNKS[c]
                g.wait_ge(s_dve, c + 1)
                g.wait_ge(s_act, c + 1)
                ys = (c % YB) * N
                g.dma_start(
                    out=yt_d[cs : cs + cw, :], in_=y_s[0:cw, ys : ys + N]
                ).then_inc(s_out, 16)

    return nc


def _prepare_inputs(X, W, b):
    """Host-side shard + repack: fused fp16 tiles [NCH, 128, TW] per core.

    Returns (in_maps, s_p) where s_p [P] are the per-position uint8 output
    dequantization scales, from the saturation-free bound
    |Y[n,p]| <= sum_k |W[p,k]| * max|X| + |b_p|.
    """
    Xh = np.ascontiguousarray(X, dtype=np.float32).astype(np.float16)
    Wh = np.ascontiguousarray(W[:, :, 0], dtype=np.float32).astype(np.float16)
    bs = np.ascontiguousarray(b[:, 0], dtype=np.float32)  # [P]

    xmax = float(np.abs(Xh).max())
    bound = np.abs(Wh.astype(np.float32)).sum(axis=1) * xmax + np.abs(bs)
    s_p = np.maximum(bound, 1e-30).astype(np.float32) / 127.0
    inv_s = (1.0 / s_p).astype(np.float32)
    offs = (bs * inv_s + 128.0).astype(np.float32)

    XT = np.zeros((P + K - 1, N), np.float16)
    XT[:P] = Xh.T

    in_maps = []
    for i in range(NCORES):
        base = i * PPC
        xin = np.zeros((NCH, 128, TW), np.float16)
        bvec = np.zeros((128, 2 * NCH), np.float32)
        for c, (cs, cw) in enumerate(CHUNKS):
            p0 = base + cs
            rows = cw + K - 1
            xin[c, :rows, :XCOLS] = XT[p0 : p0 + rows]
            j = np.arange(cw)
            for k in range(K):
                xin[c, j + k, XCOLS + j] = Wh[p0 + j, k]
            bvec[:cw, c] = inv_s[p0 : p0 + cw]
            bvec[:cw, NCH + c] = offs[p0 : p0 + cw]
        in_maps.append({"xin": xin, "bvec": bvec})
    return in_maps, s_p


def _run(in_maps, trace=False):
    from concourse import bass_utils

    if "nc" not in _CACHE:
        _CACHE["nc"] = _build_bass()
    return bass_utils.run_bass_kernel_spmd(
        _CACHE["nc"], in_maps, core_ids=list(range(NCORES)), trace=trace
    )


def kernel(X, W, b):
    in_maps, s_p = _prepare_inputs(X, W, b)
    res = _run(in_maps)
    YT = np.concatenate([r["yt"] for r in res.results], axis=0)  # [P, N] u8
    YT = (YT.astype(np.float32) - 128.0) * s_p[:, None]
    return np.ascontiguousarray(YT.T)
